# revision 21
# baseline (speedup 1.0000x reference)
"""Masked-softmax attention-scores kernel for Trainium2 (Bass/Tile), 8 cores.

Computes softmax(mask_fill(QK^T/sqrt(dk)) + syntax) for
q = query @ Wq.T + bq, k = key @ Wk.T + bk, heads split from d_model.

Sharding: 8 cores = 2 batches x 4 query-row quarters; every core handles all
12 heads for its (batch, row-slice).  The host passes query/key/W already
transposed (pure layout prep), so the device kernel is:
  - project q rows + full key into head-transposed qT/kT [d_model x s]
    (f32r matmuls, fp32 accumulate; 1/sqrt(dk) folded into the qT copy),
  - per 128-row tile: comb = (mask*1e9 - 1e9) + syntax on GPSIMD,
  - per head: scores matmul (K=64) into PSUM, DVE adds comb, ACT exp with
    fused row-sum, DVE reciprocal, normalize (DVE/ACT split), DMA out.
Softmax is computed without max-subtraction: scores + syntax are O(10) here
(exp cannot overflow) and masked entries sit at ~-1e9 whose exp underflows
to exactly 0, matching the reference's -1e9 mask fill.
"""

from contextlib import ExitStack

import numpy as np

B, S, D, H = 2, 2048, 768, 12
DK = D // H
P = 128
NCORES = 8
RSPLIT = 4          # query-row splits per batch
R = S // RSPLIT     # query rows per core
NEG = -1.0e9


def build_program(S=S, D=D, H=H, R=R, mm_f32r=True, mul_act_every=3):
    """Build the per-core SPMD Bass program (same program, 8 data shards)."""
    import concourse.bacc as bacc
    import concourse.mybir as mybir
    from concourse.tile import TileContext

    f32 = mybir.dt.float32
    i32 = mybir.dt.int32
    MMDT = mybir.dt.float32r if mm_f32r else f32
    ADD = mybir.AluOpType.add
    MULT = mybir.AluOpType.mult
    EXP = mybir.ActivationFunctionType.Exp
    COPY = mybir.ActivationFunctionType.Copy

    assert D % P == 0 and S % 512 == 0 and R % P == 0 and D // H == 64
    DC = D // P      # d-model chunks (6)
    RT = R // P      # query row tiles per core (4)
    NB = S // 512    # key-position blocks (4)

    nc = bacc.Bacc(trn_type="TRN2", target_bir_lowering=False, debug=False)

    # Host passes qt/kt/w*t pre-transposed (feature dim leading).
    qt_in = nc.declare_dram_parameter("qt_in", [D, R], MMDT, isOutput=False)
    kt_in = nc.declare_dram_parameter("kt_in", [D, S], MMDT, isOutput=False)
    syn = nc.declare_dram_parameter("syn", [R, S], f32, isOutput=False)
    msk = nc.declare_dram_parameter("msk", [R, S], i32, isOutput=False)
    wqt = nc.declare_dram_parameter("wqt", [D, D], MMDT, isOutput=False)
    bq = nc.declare_dram_parameter("bq", [D], f32, isOutput=False)
    wkt = nc.declare_dram_parameter("wkt", [D, D], MMDT, isOutput=False)
    bk = nc.declare_dram_parameter("bk", [D], f32, isOutput=False)
    out = nc.declare_dram_parameter("out", [H, R, S], f32, isOutput=True)

    with ExitStack() as ctx:
        tc = ctx.enter_context(TileContext(nc))

        consts = ctx.enter_context(tc.tile_pool(name="consts", bufs=1))
        ones = consts.tile([1, 512], MMDT)

        # Persistent projected activations, head-transposed: qT/kT[dc] holds
        # d_model rows [dc*128, dc*128+128) x all s columns.
        persist = ctx.enter_context(tc.tile_pool(name="persist", bufs=1))
        qT = [persist.tile([P, R], MMDT, name=f"qT{i}", tag=f"qT{i}")
              for i in range(DC)]

        # ---------------- upfront: q projection ---------------------------
        with (
            tc.tile_pool(name="qprep", bufs=1) as qpool,
            tc.tile_pool(name="pq", bufs=2, space="PSUM") as psq_pool,
        ):
            ones_f = qpool.tile([1, 512], f32, tag="ones_f")
            nc.vector.memset(ones_f, 1.0)
            nc.vector.tensor_copy(ones, ones_f)
            bq_f = qpool.tile([1, D], f32, tag="bq_f")
            bqs = qpool.tile([1, D], MMDT, tag="bqs")
            nc.sync.dma_start(out=bq_f, in_=bq[None, :])
            nc.vector.tensor_copy(bqs, bq_f)
            qTraw = [qpool.tile([P, R], MMDT, name=f"qTraw{i}", tag=f"qTraw{i}")
                     for i in range(DC)]
            wqT = [qpool.tile([P, D], MMDT, name=f"wqT{i}", tag=f"wqT{i}")
                   for i in range(DC)]
            for fj in range(DC):
                nc.sync.dma_start(out=qTraw[fj], in_=qt_in[fj * P:(fj + 1) * P, :])
                nc.sync.dma_start(out=wqT[fj], in_=wqt[fj * P:(fj + 1) * P, :])

            for dm in range(DC):
                for rb in range(max(1, R // 512)):
                    rw = min(512, R)
                    sl = slice(rb * 512, rb * 512 + rw)
                    ps = psq_pool.tile([P, rw], f32, tag="psq")
                    for fj in range(DC):
                        nc.tensor.matmul(
                            ps, wqT[fj][:, dm * P:(dm + 1) * P], qTraw[fj][:, sl],
                            start=(fj == 0), stop=False,
                        )
                    nc.tensor.matmul(
                        ps, bqs[0:1, dm * P:(dm + 1) * P], ones[0:1, :rw],
                        start=False, stop=True,
                    )
                    nc.scalar.activation(qT[dm][:, sl], ps, COPY, bias=0.0,
                                         scale=1.0 / 8.0)

        # ---------------- main: dc-outer with JIT k projection -------------
        kraw_pool = ctx.enter_context(tc.tile_pool(name="kraw", bufs=1))
        kw_pool = ctx.enter_context(tc.tile_pool(name="kw", bufs=2))
        kb_pool = ctx.enter_context(tc.tile_pool(name="kb", bufs=1))
        ktc_pool = ctx.enter_context(tc.tile_pool(name="ktc", bufs=2))
        mskp = ctx.enter_context(tc.tile_pool(name="mskp", bufs=2))
        combp = ctx.enter_context(tc.tile_pool(name="combp", bufs=1))
        spool = ctx.enter_context(tc.tile_pool(name="spool", bufs=3))
        epool = ctx.enter_context(tc.tile_pool(name="epool", bufs=4))
        rpool = ctx.enter_context(tc.tile_pool(name="rpool", bufs=4))
        pspool = ctx.enter_context(
            tc.tile_pool(name="pspool", bufs=3, space="PSUM"))
        psk_pool = ctx.enter_context(
            tc.tile_pool(name="psk", bufs=2, space="PSUM"))

        kTraw = [kraw_pool.tile([P, S], MMDT, name=f"kTraw{i}", tag=f"kTraw{i}")
                 for i in range(DC)]
        bk_f = kb_pool.tile([1, D], f32, tag="bk_f")
        bks = kb_pool.tile([1, D], MMDT, tag="bks")
        nc.sync.dma_start(out=bk_f, in_=bk[None, :])
        nc.vector.tensor_copy(bks, bk_f)

        combs = []

        def make_comb(t):
            rows = slice(t * P, (t + 1) * P)
            msk_t = mskp.tile([P, S], i32, name=f"msk{t}", tag="msk")
            nc.sync.dma_start(out=msk_t, in_=msk[rows, :])
            comb = combp.tile([P, S], f32, name=f"comb{t}", tag=f"comb{t}")
            nc.gpsimd.tensor_scalar(comb, msk_t, 1.0e9, NEG, op0=MULT, op1=ADD)
            nc.gpsimd.dma_start(out=comb, in_=syn[rows, :], accum_op=ADD)
            return comb

        # interleave key-chunk loads with mask loads / comb builds
        for nb in range(NB):
            for fj in range(DC):
                cols = slice(nb * 512, (nb + 1) * 512)
                nc.sync.dma_start(out=kTraw[fj][:, cols],
                                  in_=kt_in[fj * P:(fj + 1) * P, cols])
            combs.append(make_comb(nb))

        for dc in range(DC):
            # JIT: project kT slice for this head pair
            kTc = ktc_pool.tile([P, S], MMDT, name=f"kTc{dc}", tag="kTc")
            wk_dc = [kw_pool.tile([P, P], MMDT, name=f"wk{dc}_{fj}",
                                  tag=f"wk{fj}") for fj in range(DC)]
            for fj in range(DC):
                nc.sync.dma_start(
                    out=wk_dc[fj],
                    in_=wkt[fj * P:(fj + 1) * P, dc * P:(dc + 1) * P])
            for nb in range(NB):
                cols = slice(nb * 512, (nb + 1) * 512)
                psk = psk_pool.tile([P, 512], f32, name=f"psk{dc}_{nb}",
                                    tag="psk")
                for fj in range(DC):
                    nc.tensor.matmul(
                        psk, wk_dc[fj], kTraw[fj][:, cols],
                        start=(fj == 0), stop=False,
                    )
                nc.tensor.matmul(
                    psk, bks[0:1, dc * P:(dc + 1) * P], ones,
                    start=False, stop=True,
                )
                nc.scalar.activation(kTc[:, cols], psk, COPY, bias=0.0,
                                     scale=1.0)

            for t in range(RT):
                rows = slice(t * P, (t + 1) * P)
                comb = combs[t]
                # both heads of this dc packed into the PE array (row groups
                # 0-1 / 2-3 run concurrently, K=64 each)
                s_pair = [spool.tile([P, S], f32, name=f"s{t}_{2*dc+hh}",
                                     tag="s") for hh in range(2)]
                psw = min(1024, S)
                for half in range(S // psw):
                    hc = slice(half * psw, (half + 1) * psw)
                    pss = [pspool.tile([P, psw], f32,
                                       name=f"ps{t}_{2*dc+hh}_{half}",
                                       tag="ps") for hh in range(2)]
                    for n2 in range(psw // 512):
                        cols = slice(half * psw + n2 * 512,
                                     half * psw + (n2 + 1) * 512)
                        for hh in range(2):
                            off = 64 * hh
                            nc.tensor.matmul(
                                pss[hh][:, n2 * 512:(n2 + 1) * 512],
                                qT[dc][off:off + 64, rows],
                                kTc[off:off + 64, cols],
                                start=True, stop=True,
                                tile_position=(off, 0),
                            )
                    for hh in range(2):
                        nc.vector.tensor_add(s_pair[hh][:, hc], pss[hh],
                                             comb[:, hc])
                for hh in range(2):
                    h = 2 * dc + hh
                    s_t = s_pair[hh]
                    rowsum = rpool.tile([P, 1], f32, name=f"rs{t}_{h}",
                                        tag="rs")
                    e = epool.tile([P, S], f32, name=f"e{t}_{h}", tag="e")
                    nc.scalar.activation(e, s_t, EXP, accum_out=rowsum)
                    rrec = rpool.tile([P, 1], f32, name=f"rr{t}_{h}", tag="rr")
                    nc.vector.reciprocal(rrec, rowsum)
                    if h % mul_act_every == mul_act_every - 1:
                        nc.scalar.activation(e, e, COPY, bias=0.0, scale=rrec)
                    else:
                        nc.vector.tensor_scalar(e, e, scalar1=rrec,
                                                scalar2=None, op0=MULT)
                    nc.sync.dma_start(out=out[h, rows, :], in_=e)

    nc.finalize()
    return nc


_NC_CACHE = {}


def _get_nc():
    key = "full"
    if key not in _NC_CACHE:
        _NC_CACHE[key] = build_program()
    return _NC_CACHE[key]


def shard_inputs(query, key, syntax_matrix, mask, Wq, bq, Wk, bk):
    wqt = np.ascontiguousarray(Wq.T, np.float32)
    wkt = np.ascontiguousarray(Wk.T, np.float32)
    bq = np.ascontiguousarray(bq, np.float32)
    bk = np.ascontiguousarray(bk, np.float32)
    in_maps = []
    for c in range(NCORES):
        b, r = divmod(c, RSPLIT)
        rows = slice(r * R, (r + 1) * R)
        in_maps.append({
            "qt_in": np.ascontiguousarray(query[b, rows, :].T, np.float32),
            "kt_in": np.ascontiguousarray(key[b].T, np.float32),
            "syn": np.ascontiguousarray(syntax_matrix[b, 0, rows, :], np.float32),
            "msk": np.ascontiguousarray(mask[b, rows, :], np.int32),
            "wqt": wqt,
            "bq": bq,
            "wkt": wkt,
            "bk": bk,
        })
    return in_maps


def assemble_output(results):
    out = np.empty((B, H, S, S), np.float32)
    for c in range(NCORES):
        b, r = divmod(c, RSPLIT)
        out[b, :, r * R:(r + 1) * R, :] = results[c]["out"]
    return out


def run_spmd(in_maps, **kwargs):
    from concourse.bass_utils import run_bass_kernel_spmd

    nc = _get_nc()
    return run_bass_kernel_spmd(nc, in_maps, list(range(NCORES)), **kwargs)


def kernel(query, key, vm, syntax_matrix, mask, Wq, bq, Wk, bk):
    query = np.asarray(query, np.float32)
    key = np.asarray(key, np.float32)
    syntax_matrix = np.asarray(syntax_matrix, np.float32)
    mask = np.asarray(mask, np.int32)
    Wq = np.asarray(Wq, np.float32)
    bq = np.asarray(bq, np.float32)
    Wk = np.asarray(Wk, np.float32)
    bk = np.asarray(bk, np.float32)

    in_maps = shard_inputs(query, key, syntax_matrix, mask, Wq, bq, Wk, bk)
    res = run_spmd(in_maps)
    return assemble_output(res.results)


# revision 22
# speedup vs baseline: 1.1581x; 1.1581x over previous
"""Masked-softmax attention-scores kernel for Trainium2 (Bass/Tile), 8 cores.

Computes softmax(mask_fill(QK^T/sqrt(dk)) + syntax) for
q = query @ Wq.T + bq, k = key @ Wk.T + bk, heads split from d_model.

Sharding: 8 cores = 2 batches x 4 query-row quarters; every core handles all
12 heads for its (batch, row-slice).  The host passes query/key/W already
transposed (pure layout prep), so the device kernel is:
  - project q rows + full key into head-transposed qT/kT [d_model x s]
    (f32r matmuls, fp32 accumulate; 1/sqrt(dk) folded into the qT copy),
  - per 128-row tile: comb = (mask*1e9 - 1e9) + syntax on GPSIMD,
  - per head: scores matmul (K=64) into PSUM, DVE adds comb, ACT exp with
    fused row-sum, DVE reciprocal, normalize (DVE/ACT split), DMA out.
Softmax is computed without max-subtraction: scores + syntax are O(10) here
(exp cannot overflow) and masked entries sit at ~-1e9 whose exp underflows
to exactly 0, matching the reference's -1e9 mask fill.
"""

from contextlib import ExitStack

import numpy as np

B, S, D, H = 2, 2048, 768, 12
DK = D // H
P = 128
NCORES = 8
RSPLIT = 4          # query-row splits per batch
R = S // RSPLIT     # query rows per core
NEG = -1.0e9


def build_program(S=S, D=D, H=H, R=R, mm_f32r=True, mul_act_every=3):
    """Build the per-core SPMD Bass program (same program, 8 data shards)."""
    import concourse.bacc as bacc
    import concourse.mybir as mybir
    from concourse.tile import TileContext

    f32 = mybir.dt.float32
    i32 = mybir.dt.int32
    MMDT = mybir.dt.float32r if mm_f32r else f32
    ADD = mybir.AluOpType.add
    MULT = mybir.AluOpType.mult
    EXP = mybir.ActivationFunctionType.Exp
    COPY = mybir.ActivationFunctionType.Copy

    assert D % P == 0 and S % 512 == 0 and R % P == 0 and D // H == 64
    DC = D // P      # d-model chunks (6)
    RT = R // P      # query row tiles per core (4)
    NB = S // 512    # key-position blocks (4)

    nc = bacc.Bacc(trn_type="TRN2", target_bir_lowering=False, debug=False)

    # Host passes qt/kt/w*t pre-transposed (feature dim leading).
    qt_in = nc.declare_dram_parameter("qt_in", [D, R], MMDT, isOutput=False)
    kt_in = nc.declare_dram_parameter("kt_in", [D, S], MMDT, isOutput=False)
    syn = nc.declare_dram_parameter("syn", [R, S], f32, isOutput=False)
    msk = nc.declare_dram_parameter("msk", [R, S], i32, isOutput=False)
    wqt = nc.declare_dram_parameter("wqt", [D, D], MMDT, isOutput=False)
    bq = nc.declare_dram_parameter("bq", [D], f32, isOutput=False)
    wkt = nc.declare_dram_parameter("wkt", [D, D], MMDT, isOutput=False)
    bk = nc.declare_dram_parameter("bk", [D], f32, isOutput=False)
    out = nc.declare_dram_parameter("out", [H, R, S], f32, isOutput=True)

    with ExitStack() as ctx:
        tc = ctx.enter_context(TileContext(nc))

        consts = ctx.enter_context(tc.tile_pool(name="consts", bufs=1))
        ones = consts.tile([1, 512], MMDT)

        # Persistent projected activations, head-transposed: qT/kT[dc] holds
        # d_model rows [dc*128, dc*128+128) x all s columns.
        persist = ctx.enter_context(tc.tile_pool(name="persist", bufs=1))
        qT = [persist.tile([P, R], MMDT, name=f"qT{i}", tag=f"qT{i}")
              for i in range(DC)]

        # ---------------- upfront: q projection ---------------------------
        with (
            tc.tile_pool(name="qprep", bufs=1) as qpool,
            tc.tile_pool(name="pq", bufs=2, space="PSUM") as psq_pool,
        ):
            ones_f = qpool.tile([1, 512], f32, tag="ones_f")
            nc.vector.memset(ones_f, 1.0)
            nc.vector.tensor_copy(ones, ones_f)
            bq_f = qpool.tile([1, D], f32, tag="bq_f")
            bqs = qpool.tile([1, D], MMDT, tag="bqs")
            nc.sync.dma_start(out=bq_f, in_=bq[None, :])
            nc.vector.tensor_copy(bqs, bq_f)
            qTraw = [qpool.tile([P, R], MMDT, name=f"qTraw{i}", tag=f"qTraw{i}")
                     for i in range(DC)]
            wqT = [qpool.tile([P, D], MMDT, name=f"wqT{i}", tag=f"wqT{i}")
                   for i in range(DC)]
            for fj in range(DC):
                nc.sync.dma_start(out=qTraw[fj], in_=qt_in[fj * P:(fj + 1) * P, :])
                nc.sync.dma_start(out=wqT[fj], in_=wqt[fj * P:(fj + 1) * P, :])

            for dm in range(DC):
                for rb in range(max(1, R // 512)):
                    rw = min(512, R)
                    sl = slice(rb * 512, rb * 512 + rw)
                    ps = psq_pool.tile([P, rw], f32, tag="psq")
                    for fj in range(DC):
                        nc.tensor.matmul(
                            ps, wqT[fj][:, dm * P:(dm + 1) * P], qTraw[fj][:, sl],
                            start=(fj == 0), stop=False,
                        )
                    nc.tensor.matmul(
                        ps, bqs[0:1, dm * P:(dm + 1) * P], ones[0:1, :rw],
                        start=False, stop=True,
                    )
                    nc.scalar.activation(qT[dm][:, sl], ps, COPY, bias=0.0,
                                         scale=1.0 / 8.0)

        # ---------------- main: dc-outer with JIT k projection -------------
        kraw_pool = ctx.enter_context(tc.tile_pool(name="kraw", bufs=1))
        kw_pool = ctx.enter_context(tc.tile_pool(name="kw", bufs=2))
        kb_pool = ctx.enter_context(tc.tile_pool(name="kb", bufs=1))
        ktc_pool = ctx.enter_context(tc.tile_pool(name="ktc", bufs=2))
        mskp = ctx.enter_context(tc.tile_pool(name="mskp", bufs=2))
        combp = ctx.enter_context(tc.tile_pool(name="combp", bufs=1))
        spool = ctx.enter_context(tc.tile_pool(name="spool", bufs=2))
        epool = ctx.enter_context(tc.tile_pool(name="epool", bufs=3))
        rpool = ctx.enter_context(tc.tile_pool(name="rpool", bufs=4))
        pspool = ctx.enter_context(
            tc.tile_pool(name="pspool", bufs=2, space="PSUM"))
        psk_pool = ctx.enter_context(
            tc.tile_pool(name="psk", bufs=2, space="PSUM"))

        kTraw = [kraw_pool.tile([P, S], MMDT, name=f"kTraw{i}", tag=f"kTraw{i}")
                 for i in range(DC)]
        bk_f = kb_pool.tile([1, D], f32, tag="bk_f")
        bks = kb_pool.tile([1, D], MMDT, tag="bks")
        nc.sync.dma_start(out=bk_f, in_=bk[None, :])
        nc.vector.tensor_copy(bks, bk_f)

        combs = []

        def make_comb(t):
            rows = slice(t * P, (t + 1) * P)
            msk_t = mskp.tile([P, S], i32, name=f"msk{t}", tag="msk")
            nc.sync.dma_start(out=msk_t, in_=msk[rows, :])
            comb = combp.tile([P, S], f32, name=f"comb{t}", tag=f"comb{t}")
            nc.gpsimd.tensor_scalar(comb, msk_t, 1.0e9, NEG, op0=MULT, op1=ADD)
            nc.gpsimd.dma_start(out=comb, in_=syn[rows, :], accum_op=ADD)
            return comb

        # interleave key-chunk loads with mask loads / comb builds
        for nb in range(NB):
            for fj in range(DC):
                cols = slice(nb * 512, (nb + 1) * 512)
                nc.sync.dma_start(out=kTraw[fj][:, cols],
                                  in_=kt_in[fj * P:(fj + 1) * P, cols])
            combs.append(make_comb(nb))

        for dc in range(DC):
            # JIT: project kT slice for this head pair
            kTc = ktc_pool.tile([P, S], MMDT, name=f"kTc{dc}", tag="kTc")
            wk_dc = [kw_pool.tile([P, P], MMDT, name=f"wk{dc}_{fj}",
                                  tag=f"wk{fj}") for fj in range(DC)]
            for fj in range(DC):
                nc.sync.dma_start(
                    out=wk_dc[fj],
                    in_=wkt[fj * P:(fj + 1) * P, dc * P:(dc + 1) * P])
            for nb in range(NB):
                cols = slice(nb * 512, (nb + 1) * 512)
                psk = psk_pool.tile([P, 512], f32, name=f"psk{dc}_{nb}",
                                    tag="psk")
                for fj in range(DC):
                    nc.tensor.matmul(
                        psk, wk_dc[fj], kTraw[fj][:, cols],
                        start=(fj == 0), stop=False,
                    )
                nc.tensor.matmul(
                    psk, bks[0:1, dc * P:(dc + 1) * P], ones,
                    start=False, stop=True,
                )
                nc.scalar.activation(kTc[:, cols], psk, COPY, bias=0.0,
                                     scale=1.0)

            for t in range(RT):
                rows = slice(t * P, (t + 1) * P)
                comb = combs[t]
                # both heads of this dc packed into the PE array (row groups
                # 0-1 / 2-3 run concurrently, K=64 each)
                s_pair = [spool.tile([P, S], f32, name=f"s{t}_{2*dc+hh}",
                                     tag="s") for hh in range(2)]
                psw = min(1024, S)
                for half in range(S // psw):
                    hc = slice(half * psw, (half + 1) * psw)
                    pss = [pspool.tile([P, psw], f32,
                                       name=f"ps{t}_{2*dc+hh}_{half}",
                                       tag="ps") for hh in range(2)]
                    for n2 in range(psw // 512):
                        cols = slice(half * psw + n2 * 512,
                                     half * psw + (n2 + 1) * 512)
                        for hh in range(2):
                            off = 64 * hh
                            nc.tensor.matmul(
                                pss[hh][:, n2 * 512:(n2 + 1) * 512],
                                qT[dc][off:off + 64, rows],
                                kTc[off:off + 64, cols],
                                start=True, stop=True,
                                tile_position=(off, 0),
                            )
                    for hh in range(2):
                        nc.vector.tensor_add(s_pair[hh][:, hc], pss[hh],
                                             comb[:, hc])
                for hh in range(2):
                    h = 2 * dc + hh
                    s_t = s_pair[hh]
                    rowsum = rpool.tile([P, 1], f32, name=f"rs{t}_{h}",
                                        tag="rs")
                    e = epool.tile([P, S], f32, name=f"e{t}_{h}", tag="e")
                    nc.scalar.activation(e, s_t, EXP, accum_out=rowsum)
                    rrec = rpool.tile([P, 1], f32, name=f"rr{t}_{h}", tag="rr")
                    nc.vector.reciprocal(rrec, rowsum)
                    if h % mul_act_every == mul_act_every - 1:
                        nc.scalar.activation(e, e, COPY, bias=0.0, scale=rrec)
                    else:
                        nc.vector.tensor_scalar(e, e, scalar1=rrec,
                                                scalar2=None, op0=MULT)
                    nc.scalar.dma_start(out=out[h, rows, :], in_=e)

    nc.finalize()
    return nc


_NC_CACHE = {}


def _get_nc():
    key = "full"
    if key not in _NC_CACHE:
        _NC_CACHE[key] = build_program()
    return _NC_CACHE[key]


def shard_inputs(query, key, syntax_matrix, mask, Wq, bq, Wk, bk):
    wqt = np.ascontiguousarray(Wq.T, np.float32)
    wkt = np.ascontiguousarray(Wk.T, np.float32)
    bq = np.ascontiguousarray(bq, np.float32)
    bk = np.ascontiguousarray(bk, np.float32)
    in_maps = []
    for c in range(NCORES):
        b, r = divmod(c, RSPLIT)
        rows = slice(r * R, (r + 1) * R)
        in_maps.append({
            "qt_in": np.ascontiguousarray(query[b, rows, :].T, np.float32),
            "kt_in": np.ascontiguousarray(key[b].T, np.float32),
            "syn": np.ascontiguousarray(syntax_matrix[b, 0, rows, :], np.float32),
            "msk": np.ascontiguousarray(mask[b, rows, :], np.int32),
            "wqt": wqt,
            "bq": bq,
            "wkt": wkt,
            "bk": bk,
        })
    return in_maps


def assemble_output(results):
    out = np.empty((B, H, S, S), np.float32)
    for c in range(NCORES):
        b, r = divmod(c, RSPLIT)
        out[b, :, r * R:(r + 1) * R, :] = results[c]["out"]
    return out


def run_spmd(in_maps, **kwargs):
    from concourse.bass_utils import run_bass_kernel_spmd

    nc = _get_nc()
    return run_bass_kernel_spmd(nc, in_maps, list(range(NCORES)), **kwargs)


def kernel(query, key, vm, syntax_matrix, mask, Wq, bq, Wk, bk):
    query = np.asarray(query, np.float32)
    key = np.asarray(key, np.float32)
    syntax_matrix = np.asarray(syntax_matrix, np.float32)
    mask = np.asarray(mask, np.int32)
    Wq = np.asarray(Wq, np.float32)
    bq = np.asarray(bq, np.float32)
    Wk = np.asarray(Wk, np.float32)
    bk = np.asarray(bk, np.float32)

    in_maps = shard_inputs(query, key, syntax_matrix, mask, Wq, bq, Wk, bk)
    res = run_spmd(in_maps)
    return assemble_output(res.results)
